# revision 5
# baseline (speedup 1.0000x reference)
"""HQQ-quantized linear + LoRA (nn_HQQLinearLoRA) on 8 trn2 NeuronCores.

  out = x @ ((W_q - zero)*scale)^T + (x @ lora_A @ lora_B) * 2.0 + bias

Sharding: 4 token-groups (batch dim) x 2 out-feature-groups = 8 cores.
Each core computes out[b, :, og*2048:(og+1)*2048] for its (b, og).

Host passes pre-transposed views (layout-only transforms):
  - xT   [4096, 2048] f32 : x[b].T, rows permuted so each 128-row k-tile's
         partition p maps to quant-group g = p % 64 (uniform across tiles)
  - wqT  [4096, 2048] i32 : W_q[o-shard].T with the same row permutation
  - scaleT/zeroT [64, 2048] f32, loraA [4096,16] (permuted), loraB [16,2048],
    bias [1,2048]

Device: dequant W on DVE using a [128, 2048] scale/zero tile (valid for every
k-tile thanks to the permutation), cast x to fp16 in-flight via SWDGE DMA,
fp16 matmul with fp32 PSUM accumulation; LoRA + bias fold into the same PSUM
accumulation as one K=17 matmul.
"""

import sys

import numpy as np

sys.path.insert(0, "/opt/trn_rl_repo")

import concourse.bass as bass  # noqa: E402
import concourse.mybir as mybir  # noqa: E402
import concourse.tile as tile  # noqa: E402
from concourse import bacc  # noqa: E402
from concourse.bass_utils import run_bass_kernel_spmd  # noqa: E402

B, S, I, O, R = 4, 2048, 4096, 4096, 16
GS = 64
G = I // GS  # 64
NCORES = 8
OG = 2
O_SH = O // OG  # 2048
T = S  # 2048 tokens per core
KT = I // 128  # 32 k-tiles
TCH = 512  # token chunk
NTCH = T // TCH  # 4
OCH = 512  # o quarter (dequant-W granule)
NOCH = O_SH // OCH  # 4
SCALING = 2.0

F32 = mybir.dt.float32
F16 = mybir.dt.float16
I32 = mybir.dt.int32

TRACE = False
TRACE_KWARGS = {}
LAST_RESULTS = None


def _perm() -> np.ndarray:
    """Row order such that k-tile k, partition p holds input-feature
    i = (p % 64)*64 + 2k + p//64, i.e. quant group g(i) = p % 64."""
    p = np.arange(128)
    out = np.empty(I, dtype=np.int64)
    for k in range(KT):
        out[k * 128 + p] = (p % 64) * 64 + 2 * k + p // 64
    return out


PERM = _perm()

_nc_cache = None


def _build():
    nc = bacc.Bacc(None)
    xT_d = nc.dram_tensor("xT", [I, T], F32, kind="ExternalInput")
    wqT_d = nc.dram_tensor("wqT", [I, O_SH], I32, kind="ExternalInput")
    scaleT_d = nc.dram_tensor("scaleT", [G, O_SH], F32, kind="ExternalInput")
    zeroT_d = nc.dram_tensor("zeroT", [G, O_SH], F32, kind="ExternalInput")
    loraA_d = nc.dram_tensor("loraA", [I, R], F32, kind="ExternalInput")
    loraB_d = nc.dram_tensor("loraB", [R, O_SH], F32, kind="ExternalInput")
    bias_d = nc.dram_tensor("bias", [1, O_SH], F32, kind="ExternalInput")
    ones_d = nc.dram_tensor("ones", [1, T], F32, kind="ExternalInput")
    out_d = nc.dram_tensor("out", [T, O_SH], F32, kind="ExternalOutput")

    Copy = mybir.ActivationFunctionType.Copy

    with tile.TileContext(nc) as tc:
        with (
            tc.tile_pool(name="const", bufs=1) as constp,
            tc.tile_pool(name="w16", bufs=3) as w16p,
            tc.tile_pool(name="wq", bufs=3) as wqp,
            tc.tile_pool(name="d16", bufs=2) as d16p,
            tc.tile_pool(name="x16", bufs=2) as x16p,
            tc.tile_pool(name="ob", bufs=3) as obp,
            tc.tile_pool(name="ps", bufs=4, space="PSUM") as psp,
            tc.tile_pool(name="psl", bufs=2, space="PSUM") as pslp,
        ):
            # ---- constants ----
            # scale/zero expanded: row p <- scaleT[p % 64, :]
            s16 = constp.tile([128, O_SH], F16)
            z16 = constp.tile([128, O_SH], F16)
            for h in (0, 1):
                nc.gpsimd.dma_start(s16[64 * h : 64 * h + 64, :], scaleT_d[:, :])
                nc.gpsimd.dma_start(z16[64 * h : 64 * h + 64, :], zeroT_d[:, :])
            # lora_A tiles: [128, (k r)]; fold the 2.0 LoRA scaling here
            laf = constp.tile([128, KT, R], F32)
            nc.sync.dma_start(laf[:], loraA_d.rearrange("(k p) r -> p k r", p=128))
            la16 = constp.tile([128, KT, R], F16)
            nc.scalar.activation(la16[:], laf[:], Copy, scale=SCALING)
            # [loraB; bias] rhs for the fused K=17 matmul
            lb16 = constp.tile([R + 1, O_SH], F16)
            nc.gpsimd.dma_start(lb16[0:R, :], loraB_d[:])
            nc.gpsimd.dma_start(lb16[R : R + 1, :], bias_d[:])
            # [t1; ones] lhsT rows; row R stays 1.0
            t1sb = constp.tile([R + 1, T], F16)
            nc.gpsimd.dma_start(t1sb[R : R + 1, :], ones_d[:])

            for oh in range(2):  # o-halves (x streamed once per half)
                w16q_list = []
                for qq in range(2):  # dequant W in o-quarters of 512
                    oq = oh * 2 + qq
                    w16q = w16p.tile([128, KT * OCH], F16)
                    for k in range(KT):
                        wq16 = wqp.tile([128, OCH], F16)
                        nc.gpsimd.dma_start(
                            wq16[:],
                            wqT_d[k * 128 : (k + 1) * 128, oq * OCH : (oq + 1) * OCH],
                        )
                        d16 = d16p.tile([128, OCH], F16)
                        nc.vector.tensor_sub(
                            d16[:], wq16[:], z16[:, oq * OCH : (oq + 1) * OCH]
                        )
                        nc.vector.tensor_mul(
                            w16q[:, k * OCH : (k + 1) * OCH],
                            d16[:],
                            s16[:, oq * OCH : (oq + 1) * OCH],
                        )
                    w16q_list.append(w16q)

                for tci in range(NTCH):
                    # one 8 MiB casting DMA per token chunk: f32 -> f16
                    x16 = x16p.tile([128, KT, TCH], F16)
                    nc.gpsimd.dma_start(
                        x16[:],
                        xT_d[:, tci * TCH : (tci + 1) * TCH].rearrange(
                            "(k p) t -> p k t", p=128
                        ),
                    )
                    if oh == 0:
                        t1ps = pslp.tile([R, TCH], F32)
                        for k in range(KT):
                            nc.tensor.matmul(
                                t1ps[:],
                                la16[:, k, :],
                                x16[:, k, :],
                                start=(k == 0),
                                stop=(k == KT - 1),
                            )
                        nc.vector.tensor_copy(
                            t1sb[0:R, tci * TCH : (tci + 1) * TCH], t1ps[:]
                        )
                    for tt in range(TCH // 128):
                        t0 = tci * TCH + tt * 128
                        for qq in range(2):
                            o0 = (oh * 2 + qq) * OCH
                            ps = psp.tile([128, OCH], F32)
                            for k in range(KT):
                                nc.tensor.matmul(
                                    ps[:],
                                    x16[:, k, tt * 128 : tt * 128 + 128],
                                    w16q_list[qq][:, k * OCH : (k + 1) * OCH],
                                    start=(k == 0),
                                    stop=False,
                                )
                            nc.tensor.matmul(
                                ps[:],
                                t1sb[:, t0 : t0 + 128],
                                lb16[:, o0 : o0 + OCH],
                                start=False,
                                stop=True,
                            )
                            ob = obp.tile([128, OCH], F32)
                            nc.vector.tensor_copy(ob[:], ps[:])
                            nc.sync.dma_start(out_d[t0 : t0 + 128, o0 : o0 + OCH], ob[:])

    nc.compile()
    return nc


def kernel(x, W_q, scale, zero, lora_A, lora_B, bias):
    global _nc_cache, LAST_RESULTS
    if _nc_cache is None:
        _nc_cache = _build()
    nc = _nc_cache

    x = np.asarray(x, dtype=np.float32)
    W_q = np.asarray(W_q, dtype=np.int32)
    scale = np.asarray(scale, dtype=np.float32)
    zero = np.asarray(zero, dtype=np.float32)
    lora_A = np.asarray(lora_A, dtype=np.float32)
    lora_B = np.asarray(lora_B, dtype=np.float32)
    bias = np.asarray(bias, dtype=np.float32)

    loraA_p = np.ascontiguousarray(lora_A[PERM])
    # xT per batch element (shared by the 2 o-group cores)
    xT_b = [np.ascontiguousarray(x[b].T[PERM]) for b in range(B)]

    in_maps = []
    for c in range(NCORES):
        b, og = c // OG, c % OG
        osl = slice(og * O_SH, (og + 1) * O_SH)
        in_maps.append(
            {
                "xT": xT_b[b],
                "wqT": np.ascontiguousarray(W_q[osl].T[PERM]),
                "scaleT": np.ascontiguousarray(scale[osl].T),
                "zeroT": np.ascontiguousarray(zero[osl].T),
                "loraA": loraA_p,
                "loraB": np.ascontiguousarray(lora_B[:, osl]),
                "bias": np.ascontiguousarray(bias[osl]).reshape(1, O_SH),
                "ones": np.ones((1, T), dtype=np.float32),
            }
        )

    res = run_bass_kernel_spmd(
        nc,
        in_maps,
        core_ids=list(range(NCORES)),
        trace=TRACE,
        trace_kwargs=TRACE_KWARGS,
    )
    LAST_RESULTS = res

    out = np.empty((B, S, O), dtype=np.float32)
    for c in range(NCORES):
        b, og = c // OG, c % OG
        out[b, :, og * O_SH : (og + 1) * O_SH] = res.results[c]["out"]
    return out
